# revision 1
# baseline (speedup 1.0000x reference)
"""Trainium2 Bass kernel for nn_AttentionBlock (sparse attention with gaussian bias).

Reference computation (per batch b):
    qp = q @ Wq + bq; kp = k @ Wk + bk; vp = v @ Wv + bv          (d_model=512 -> dk=dv=64)
    attn = qp @ kp^T / 8 + g_bias / (2 tau^2); attn[mask] = -inf
    p = softmax(attn, axis=-1)
    out = (p @ vp) @ Wfc + bfc

Sharding: 8 cores = (batch b in 0..3) x (query-half h in 0..1).
Each core computes a [1024, 2048] attention slab. K/V work is split within each
core pair: each core transposes+projects its half of K/V, then kpT / vp are
AllGathered over the pair (small projected tensors instead of raw K/V).

Per-core dataflow (Sq=1024 local, Sk=2048):
  Phase A: PE-transpose q and half of k/v, project:
      qpT[64,1024] = Wq^T qT * (2 tau^2/8) + bq',  kpT_half[64,1024] = Wk^T kT + bk,
      vp_half[1024,64] = v Wv + bv;  AllGather kpT, vp across the pair.
  Phase B per sq-tile [128 rows]:
      psum = qpT^T @ kpT  (+ I_r @ gm accumulate, gm = g_bias - 1e30*mask, f32r)
      e = exp(psum / (2 tau^2)) with row-sum accumulator (ACT, f32r out)
      eT via PE transposes; unnormalized oT[64,sq] = sum_k vp[k,:]^T e[:,k]
      out = (oT^T @ Wfc) * (1/rowsum) + bfc
"""
import numpy as np

B, S, D, DKV = 4, 2048, 512, 64
SQL = S // 2          # query rows per core
SKL = S // 2          # k/v rows loaded per core (pair-sharded)
N_CORES = 8
NT_K = S // 128       # 16 k/v tiles (full)
NG_Q = SQL // 512     # 2 groups of 4 q-tiles
NG_KL = SKL // 512    # 2 groups of local k/v rows

PAIR_KV = True        # split K/V across core pairs + AllGather projections


def _build():
    import concourse.bass as bass
    import concourse.mybir as mybir
    import concourse.tile as tile
    from concourse import bacc

    f32, bf16, u8 = mybir.dt.float32, mybir.dt.bfloat16, mybir.dt.uint8
    f16 = mybir.dt.float16
    f32r = mybir.dt.float32r
    AF = mybir.ActivationFunctionType
    OP = mybir.AluOpType

    nc = bacc.Bacc(num_devices=N_CORES)
    skl = SKL if PAIR_KV else S
    q_ext = nc.declare_dram_parameter("q", [SQL, D], f32, isOutput=False)
    k_ext = nc.declare_dram_parameter("k", [skl, D], f32, isOutput=False)
    v_ext = nc.declare_dram_parameter("v", [skl, D], f32, isOutput=False)
    gb_ext = nc.declare_dram_parameter("gb", [SQL, S], f32, isOutput=False)
    m_ext = nc.declare_dram_parameter("mask", [SQL, S], u8, isOutput=False)
    wq_ext = nc.declare_dram_parameter("Wq", [D, DKV], f32, isOutput=False)
    wk_ext = nc.declare_dram_parameter("Wk", [D, DKV], f32, isOutput=False)
    wv_ext = nc.declare_dram_parameter("Wv", [D, DKV], f32, isOutput=False)
    wfc_ext = nc.declare_dram_parameter("Wfc", [DKV, D], f32, isOutput=False)
    bq_ext = nc.declare_dram_parameter("bq", [DKV, 1], f32, isOutput=False)
    bk_ext = nc.declare_dram_parameter("bk", [DKV, 1], f32, isOutput=False)
    bv_ext = nc.declare_dram_parameter("bvb", [128, DKV], f32, isOutput=False)
    bfc_ext = nc.declare_dram_parameter("bfcb", [128, D], f32, isOutput=False)
    # host-derived scalars: qscale = 2*tau^2/8 (per dk partition), escale = 1/(2 tau^2)
    qs_ext = nc.declare_dram_parameter("qscale", [DKV, 1], f32, isOutput=False)
    es_ext = nc.declare_dram_parameter("escale", [128, 1], f32, isOutput=False)
    out_ext = nc.declare_dram_parameter("out", [SQL, D], f32, isOutput=True)

    # collective bounce buffers (internal DRAM; outs in Shared space)
    if PAIR_KV:
        kp_ag_in = nc.dram_tensor("kp_ag_in", [DKV, SKL], f32r)
        kp_ag_out = nc.dram_tensor("kp_ag_out", [2, DKV, SKL], f32r)
        vp_ag_in = nc.dram_tensor("vp_ag_in", [128, NT_K // 2, DKV], mybir.dt.float16)
        vp_ag_out = nc.dram_tensor("vp_ag_out", [2, 128, NT_K // 2, DKV], mybir.dt.float16)
        pair_groups = [[2 * b, 2 * b + 1] for b in range(4)]

    with tile.TileContext(nc) as tc:
        from contextlib import ExitStack
        with ExitStack() as ctx:
            wpool = ctx.enter_context(tc.tile_pool(name="weights", bufs=1))
            proj_pool = ctx.enter_context(tc.tile_pool(name="proj", bufs=1))

            # ---- small weights / constants ----
            wq_t = wpool.tile([128, 4, DKV], f32, tag="wq")
            wk_t = wpool.tile([128, 4, DKV], f32, tag="wk")
            wv_t = wpool.tile([128, 4, DKV], f32, tag="wv")
            nc.sync.dma_start(wq_t[:], wq_ext.rearrange("(c p) n -> p c n", p=128))
            nc.sync.dma_start(wk_t[:], wk_ext.rearrange("(c p) n -> p c n", p=128))
            nc.sync.dma_start(wv_t[:], wv_ext.rearrange("(c p) n -> p c n", p=128))
            wfc_t = wpool.tile([DKV, D], f32, tag="wfc")
            nc.sync.dma_start(wfc_t[:], wfc_ext[:])
            bq_t = wpool.tile([DKV, 1], f32, tag="bq")
            bk_t = wpool.tile([DKV, 1], f32, tag="bk")
            bv_t = wpool.tile([128, DKV], f32, tag="bv")
            bfc_t = wpool.tile([128, D], f32, tag="bfc")
            qs_t = wpool.tile([DKV, 1], f32, tag="qs")
            es_t = wpool.tile([128, 1], f32, tag="es")
            nc.sync.dma_start(bq_t[:], bq_ext[:])
            nc.sync.dma_start(bk_t[:], bk_ext[:])
            nc.sync.dma_start(bv_t[:], bv_ext[:])
            nc.sync.dma_start(bfc_t[:], bfc_ext[:])
            nc.sync.dma_start(qs_t[:], qs_ext[:])
            nc.sync.dma_start(es_t[:], es_ext[:])

            # rounded weights for matmuls
            wq_r = wpool.tile([128, 4, DKV], f32r, tag="wq_r")
            wk_r = wpool.tile([128, 4, DKV], f32r, tag="wk_r")
            wfc_r = wpool.tile([DKV, D], f32r, tag="wfc_r")
            nc.vector.tensor_copy(wq_r[:], wq_t[:])
            nc.vector.tensor_copy(wk_r[:], wk_t[:])
            nc.vector.tensor_copy(wfc_r[:], wfc_t[:])

            # identities: f32 for qkv transposes, bf16 for gm add, f16 for eT
            ident = wpool.tile([128, 128], f32, tag="ident")
            ident_bf = wpool.tile([128, 128], bf16, tag="ident_bf")
            ident_h = wpool.tile([128, 128], f16, tag="ident_h")
            from concourse.masks import make_identity
            make_identity(nc, ident[:])
            nc.vector.tensor_copy(ident_bf[:], ident[:])
            nc.vector.tensor_copy(ident_h[:], ident[:])
            eb_t = wpool.tile([128, 1], f32, tag="eb")
            nc.gpsimd.memset(eb_t[:], -3.0)

            # ---- persistent projected tensors (local half computed here, remote
            # half arrives via pair AllGather; sk axis is host-permuted so the
            # local half always occupies columns 0:1024) ----
            kpT_loc = proj_pool.tile([DKV, SKL], f32r, tag="kpT_loc")
            kpT_rem = proj_pool.tile([DKV, SKL], f32r, tag="kpT_rem")
            qpT = proj_pool.tile([DKV, SQL], f32r, tag="qpT")       # [64, 1024]
            vp_loc = proj_pool.tile([128, NT_K // 2, DKV], f16, tag="vp_loc")
            vp_rem = proj_pool.tile([128, NT_K // 2, DKV], f16, tag="vp_rem")

            with tc.tile_pool(name="pa_sbuf", bufs=4) as pa_pool, \
                 tc.tile_pool(name="pa_psumT", bufs=3, space="PSUM") as pa_psT, \
                 tc.tile_pool(name="pa_psumP", bufs=2, space="PSUM") as pa_psP:

                def load_transpose_group(x_ext, g, dt_out, tag, copy_eng, dma_eng):
                    """Load 512 rows of x (one DMA), transpose on PE.
                    Returns xT_sb [128, 4, 512]: chunk j holds xT[d_chunk_j, 512 rows]."""
                    x_t = pa_pool.tile([128, 4, D], f32, tag="x_in")
                    dma_eng(x_t[:],
                            x_ext[512 * g:512 * (g + 1), :]
                            .rearrange("(t p) d -> p t d", p=128))
                    xT_sb = pa_pool.tile([128, 4, 512], dt_out, tag=tag)
                    for t in range(4):
                        ps = pa_psT.tile([128, 4, 128], f32, tag="psT")
                        for j in range(4):
                            nc.tensor.transpose(
                                ps[:, j, :], x_t[:, t, 128 * j:128 * (j + 1)], ident[:])
                        copy_eng(xT_sb[:, :, 128 * t:128 * (t + 1)], ps[:])
                    return xT_sb

                # K local half: kpT_loc [64, SKL]
                ng_k = NG_KL
                for g in range(ng_k):
                    kT = load_transpose_group(k_ext, g, f32r, "xTr",
                                              nc.scalar.copy, nc.sync.dma_start)
                    pp = pa_psP.tile([DKV, 512], f32, tag="psP")
                    for j in range(4):
                        nc.tensor.matmul(pp[:], wk_r[:, j, :], kT[:, j, :],
                                         start=(j == 0), stop=(j == 3))
                    nc.vector.tensor_scalar(
                        out=kpT_loc[:, 512 * g:512 * (g + 1)], in0=pp[:],
                        scalar1=bk_t[:], scalar2=None, op0=OP.add)

                # exchange: send local half, fetch partner half (dynamic row)
                remote_row = 1 - (nc.sync.partition_id() % 2)
                nc.sync.dma_start(kp_ag_in[:], kpT_loc[:])
                nc.gpsimd.collective_compute(
                    "AllGather", OP.bypass, replica_groups=pair_groups,
                    ins=[kp_ag_in.ap()], outs=[kp_ag_out.ap()])
                nc.sync.dma_start(kpT_rem[:], kp_ag_out[bass.ds(remote_row, 1)].squeeze(0))

                # Q: qpT[64, 1024] scaled by 2 tau^2 / 8
                for g in range(NG_Q):
                    qT = load_transpose_group(q_ext, g, f32r, "xTr",
                                              nc.vector.tensor_copy, nc.sync.dma_start)
                    pp = pa_psP.tile([DKV, 512], f32, tag="psP")
                    for j in range(4):
                        nc.tensor.matmul(pp[:], wq_r[:, j, :], qT[:, j, :],
                                         start=(j == 0), stop=(j == 3))
                    nc.vector.tensor_scalar(
                        out=qpT[:, 512 * g:512 * (g + 1)], in0=pp[:],
                        scalar1=bq_t[:], scalar2=qs_t[:], op0=OP.add, op1=OP.mult)

                # V local half: vp natural [skl, dv], f32r, +bv
                for g in range(ng_k):
                    vT = load_transpose_group(v_ext, g, f32, "xTv",
                                              nc.scalar.copy, nc.sync.dma_start)
                    for t in range(4):
                        pv = pa_psP.tile([128, DKV], f32, tag="psV")
                        for j in range(4):
                            nc.tensor.matmul(
                                pv[:], vT[:, j, 128 * t:128 * (t + 1)], wv_t[:, j, :],
                                start=(j == 0), stop=(j == 3))
                        nc.vector.tensor_tensor(
                            out=vp_loc[:, 4 * g + t, :], in0=pv[:], in1=bv_t[:],
                            op=OP.add)

                nc.sync.dma_start(vp_ag_in[:], vp_loc[:])
                nc.gpsimd.collective_compute(
                    "AllGather", OP.bypass, replica_groups=pair_groups,
                    ins=[vp_ag_in.ap()], outs=[vp_ag_out.ap()])
                nc.sync.dma_start(vp_rem[:], vp_ag_out[bass.ds(remote_row, 1)].squeeze(0))

            # ---- phase B ----
            with tc.tile_pool(name="pb_sbuf", bufs=2) as pb_pool, \
                 tc.tile_pool(name="pb_ebuf", bufs=5) as pb_epool, \
                 tc.tile_pool(name="pb_eT", bufs=1) as pb_eTpool, \
                 tc.tile_pool(name="pb_acc", bufs=8) as pb_accpool, \
                 tc.tile_pool(name="pb_ps_s", bufs=2, space="PSUM") as pb_ps_s, \
                 tc.tile_pool(name="pb_ps_eT", bufs=2, space="PSUM") as pb_ps_eT, \
                 tc.tile_pool(name="pb_ps_pv", bufs=1, space="PSUM") as pb_ps_pv, \
                 tc.tile_pool(name="pb_ps_fc", bufs=1, space="PSUM") as pb_ps_fc:

                recips = []
                for g in range(NG_Q):
                    e_tiles = []
                    for t in range(4):
                        i = 4 * g + t
                        sq0 = 128 * i
                        gb_t = pb_pool.tile([128, S], f32, tag="gb")
                        m_bf = pb_pool.tile([128, S], bf16, tag="m")
                        nc.scalar.dma_start(gb_t[:], gb_ext[sq0:sq0 + 128, :])
                        nc.gpsimd.dma_start(m_bf[:], m_ext[sq0:sq0 + 128, :])
                        gm = pb_pool.tile([128, S], bf16, tag="gm")
                        nc.vector.scalar_tensor_tensor(
                            out=gm[:], in0=m_bf[:], scalar=-1e30, in1=gb_t[:],
                            op0=OP.mult, op1=OP.add)

                        e_bf = pb_epool.tile([128, S], f16, tag="e")
                        accs = []
                        for h, kp_half in ((0, kpT_loc), (1, kpT_rem)):
                            hs = slice(1024 * h, 1024 * (h + 1))
                            ps_s = pb_ps_s.tile([128, 1024], f32, tag="score")
                            for c in range(2):
                                sl = slice(1024 * h + 512 * c, 1024 * h + 512 * (c + 1))
                                ksl = slice(512 * c, 512 * (c + 1))
                                psl = slice(512 * c, 512 * (c + 1))
                                nc.tensor.matmul(ps_s[:, psl],
                                                 qpT[:, sq0:sq0 + 128], kp_half[:, ksl],
                                                 start=True, stop=False)
                                nc.tensor.matmul(ps_s[:, psl], ident_bf[:], gm[:, sl],
                                                 start=False, stop=True)
                            acc = pb_accpool.tile([128, 1], f32, tag=f"acc{h}")
                            nc.scalar.activation(e_bf[:, hs], ps_s[:], AF.Exp,
                                                 bias=eb_t[:], scale=es_t[:],
                                                 accum_out=acc[:])
                            accs.append(acc)
                        acc_t = pb_accpool.tile([128, 1], f32, tag="accsum")
                        nc.vector.tensor_tensor(out=acc_t[:], in0=accs[0][:],
                                                in1=accs[1][:], op=OP.add)
                        r_t = pb_accpool.tile([128, 1], f32, tag="recip")
                        nc.vector.reciprocal(r_t[:], acc_t[:])
                        recips.append(r_t)
                        e_tiles.append(e_bf)

                    # eT for the group: eT_sb[:, j, :] = e[512 rows, sk chunk j].T
                    eT_sb = pb_eTpool.tile([128, NT_K, 512], f16, tag="eT")
                    for j in range(NT_K):
                        ps_eT = pb_ps_eT.tile([128, 512], f16, tag="pseT")
                        for t in range(4):
                            nc.tensor.transpose(
                                ps_eT[:, 128 * t:128 * (t + 1)],
                                e_tiles[t][:, 128 * j:128 * (j + 1)], ident_h[:])
                        nc.vector.tensor_copy(eT_sb[:, j, :], ps_eT[:])

                    # PV: oT[64, 512] = sum_j vp_j^T @ eT_j
                    ps_pv = pb_ps_pv.tile([DKV, 512], f32, tag="pspv")
                    for j in range(NT_K):
                        vp_j = vp_loc[:, j, :] if j < NT_K // 2 else vp_rem[:, j - NT_K // 2, :]
                        nc.tensor.matmul(ps_pv[:], vp_j, eT_sb[:, j, :],
                                         start=(j == 0), stop=(j == NT_K - 1))
                    aoT = pb_pool.tile([DKV, 512], f32r, tag="aoT")
                    nc.scalar.copy(aoT[:], ps_pv[:])

                    # FC + normalize + bias + store
                    for t in range(4):
                        i = 4 * g + t
                        ps_fc = pb_ps_fc.tile([128, D], f32, tag="psfc")
                        nc.tensor.matmul(ps_fc[:], aoT[:, 128 * t:128 * (t + 1)],
                                         wfc_r[:], start=True, stop=True)
                        o_sb = pb_pool.tile([128, D], f32, tag="osb")
                        nc.vector.scalar_tensor_tensor(
                            out=o_sb[:], in0=ps_fc[:], scalar=recips[i][:],
                            in1=bfc_t[:], op0=OP.mult, op1=OP.add)
                        nc.sync.dma_start(out_ext[128 * i:128 * (i + 1), :], o_sb[:])

    nc.finalize()
    return nc


_cache = {}


def kernel(**inputs):
    from concourse.bass_utils import run_bass_kernel_spmd

    q = np.asarray(inputs["q"], np.float32)
    k = np.asarray(inputs["k"], np.float32)
    v = np.asarray(inputs["v"], np.float32)
    gb = np.asarray(inputs["g_bias"], np.float32)
    mask = np.asarray(inputs["mask"]).astype(np.uint8)
    tau = float(np.asarray(inputs["tau"]))

    if "nc" not in _cache:
        _cache["nc"] = _build()
    nc = _cache["nc"]

    in_maps = build_in_maps(inputs, q, k, v, gb, mask, tau)
    res = run_bass_kernel_spmd(nc, in_maps, list(range(N_CORES)))
    out = np.empty((B, S, D), np.float32)
    for c in range(N_CORES):
        b, h = divmod(c, 2)
        out[b, h * SQL:(h + 1) * SQL] = res.results[c]["out"]
    return out


def _perm_cols(x, h):
    """Put the core's local sk-half (columns h*1024:(h+1)*1024) first."""
    if h == 0:
        return np.ascontiguousarray(x)
    return np.ascontiguousarray(np.concatenate([x[:, SKL:], x[:, :SKL]], axis=1))


def build_in_maps(inputs, q, k, v, gb, mask, tau):
    qscale = np.full((DKV, 1), (2.0 * tau * tau) / 8.0, np.float32)
    escale = np.full((128, 1), 1.0 / (2.0 * tau * tau), np.float32)
    shared = {
        "Wq": np.asarray(inputs["Wq"], np.float32),
        "Wk": np.asarray(inputs["Wk"], np.float32),
        "Wv": np.asarray(inputs["Wv"], np.float32),
        "Wfc": np.asarray(inputs["Wfc"], np.float32),
        "bq": np.asarray(inputs["bq"], np.float32).reshape(DKV, 1).copy(),
        "bk": np.asarray(inputs["bk"], np.float32).reshape(DKV, 1).copy(),
        "bvb": np.broadcast_to(np.asarray(inputs["bv"], np.float32), (128, DKV)).copy(),
        "bfcb": np.broadcast_to(np.asarray(inputs["bfc"], np.float32), (128, D)).copy(),
        "qscale": qscale, "escale": escale,
    }
    in_maps = []
    for c in range(N_CORES):
        b, h = divmod(c, 2)
        sl = slice(h * SQL, (h + 1) * SQL)
        ksl = sl if PAIR_KV else slice(None)
        in_maps.append({
            "q": np.ascontiguousarray(q[b, sl]),
            "k": np.ascontiguousarray(k[b, ksl]),
            "v": np.ascontiguousarray(v[b, ksl]),
            "gb": _perm_cols(gb[b, sl], h),
            "mask": _perm_cols(mask[b, sl], h),
            **shared,
        })
    return in_maps



# revision 6
# speedup vs baseline: 1.8819x; 1.8819x over previous
"""Trainium2 Bass kernel for nn_AttentionBlock (sparse attention with gaussian bias).

Reference computation (per batch b):
    qp = q @ Wq + bq; kp = k @ Wk + bk; vp = v @ Wv + bv          (d_model=512 -> dk=dv=64)
    attn = qp @ kp^T / 8 + g_bias / (2 tau^2); attn[mask] = -inf
    p = softmax(attn, axis=-1)
    out = (p @ vp) @ Wfc + bfc

Sharding: 8 cores = (batch b in 0..3) x (query-half h in 0..1); K/V replicated
(no collectives). Each core computes a [1024 q, 2048 k] attention slab.

Transposed-scores design: all big operands arrive HOST-TRANSPOSED so no PE
transposes are needed anywhere:
  - qT/kT/vT [512, rows] f16; gmT [2048 k, 1024 q] fp8-e5m2 with mask folded in
    on host (masked = -57344 -> exp ~ 0).
  - qpT[64, 1024] = Wq^T qT (scaled by 2 tau^2/8, +bq'), kpT[64, 2048] = Wk^T kT
    + bk, vp[128k, 65] = (v Wv_aug + bv_aug) with ones column 64 (rowsum trick).
  - per k-tile: sT[128k, 1024q] = kpT_tile^T @ qpT  (+ ident @ gmT accumulate),
    eT[:, kt, :] = exp(sT * escale - 3)  (f16; the -3 shift cancels in softmax).
  - PV: oT_aug[65, 512q] = sum_kt vp[kt]^T... matmul(lhsT=vp[kt], rhs=eT[kt]);
    row 64 = rowsum.
  - FC per 128-q chunk: out = (aoT^T @ Wfc_aug) * recip(rowsum) + bfc, with
    rowsum extracted via a [65,1] selector matmul.
"""
import numpy as np
import ml_dtypes

B, S, D, DKV = 4, 2048, 512, 64
DV1 = DKV + 1          # vp augmented with ones column
SQL = S // 2           # query rows per core
NKT = S // 128         # 16 k tiles
N_CORES = 8
MASK_VAL = -57344.0    # max-magnitude finite fp8-e5m2; * 1/(2 tau^2) -> -31.9


def _build():
    import concourse.bass as bass
    import concourse.mybir as mybir
    import concourse.tile as tile
    from concourse import bacc

    f32, f16, f8 = mybir.dt.float32, mybir.dt.float16, mybir.dt.float8e5
    AF = mybir.ActivationFunctionType
    OP = mybir.AluOpType

    nc = bacc.Bacc(num_devices=N_CORES)
    qT_ext = nc.declare_dram_parameter("qT", [D, SQL], f16, isOutput=False)
    kT_ext = nc.declare_dram_parameter("kT", [D, S], f16, isOutput=False)
    vT_ext = nc.declare_dram_parameter("vT", [D, S], f16, isOutput=False)
    gmT_ext = nc.declare_dram_parameter("gmT", [S, SQL], f8, isOutput=False)
    wq_ext = nc.declare_dram_parameter("Wq", [D, DKV], f16, isOutput=False)
    wk_ext = nc.declare_dram_parameter("Wk", [D, DKV], f16, isOutput=False)
    wv_ext = nc.declare_dram_parameter("Wva", [D, DV1], f16, isOutput=False)
    wfc_ext = nc.declare_dram_parameter("Wfca", [DV1, D], f16, isOutput=False)
    sel_ext = nc.declare_dram_parameter("sel", [DV1, 1], f16, isOutput=False)
    bq_ext = nc.declare_dram_parameter("bq", [DKV, 1], f32, isOutput=False)
    bk_ext = nc.declare_dram_parameter("bk", [DKV, 1], f32, isOutput=False)
    bv_ext = nc.declare_dram_parameter("bvb", [128, DV1], f32, isOutput=False)
    bfc_ext = nc.declare_dram_parameter("bfcb", [128, D], f32, isOutput=False)
    # host-derived scalars: qscale = 2*tau^2/8 (per dk partition), escale = 1/(2 tau^2)
    qs_ext = nc.declare_dram_parameter("qscale", [DKV, 1], f32, isOutput=False)
    es_ext = nc.declare_dram_parameter("escale", [128, 1], f32, isOutput=False)
    out_ext = nc.declare_dram_parameter("out", [SQL, D], f32, isOutput=True)

    with tile.TileContext(nc) as tc:
        from contextlib import ExitStack
        with ExitStack() as ctx:
            wpool = ctx.enter_context(tc.tile_pool(name="weights", bufs=1))
            ppool = ctx.enter_context(tc.tile_pool(name="persist", bufs=1))

            # ---- small weights / constants ----
            wq_t = wpool.tile([128, 4, DKV], f16, tag="wq")
            wk_t = wpool.tile([128, 4, DKV], f16, tag="wk")
            wv_t = wpool.tile([128, 4, DV1], f16, tag="wv")
            wfc_t = wpool.tile([DV1, D], f16, tag="wfc")
            sel_t = wpool.tile([DV1, 1], f16, tag="sel")
            nc.sync.dma_start(wq_t[:], wq_ext.rearrange("(j p) n -> p j n", p=128))
            nc.sync.dma_start(wk_t[:], wk_ext.rearrange("(j p) n -> p j n", p=128))
            nc.sync.dma_start(wv_t[:], wv_ext.rearrange("(j p) n -> p j n", p=128))
            nc.sync.dma_start(wfc_t[:], wfc_ext[:])
            nc.sync.dma_start(sel_t[:], sel_ext[:])
            bq_t = wpool.tile([DKV, 1], f32, tag="bq")
            bk_t = wpool.tile([DKV, 1], f32, tag="bk")
            bv_t = wpool.tile([128, DV1], f32, tag="bv")
            bfc_t = wpool.tile([128, D], f32, tag="bfc")
            qs_t = wpool.tile([DKV, 1], f32, tag="qs")
            es_t = wpool.tile([128, 1], f32, tag="es")
            nc.sync.dma_start(bq_t[:], bq_ext[:])
            nc.sync.dma_start(bk_t[:], bk_ext[:])
            nc.sync.dma_start(bv_t[:], bv_ext[:])
            nc.sync.dma_start(bfc_t[:], bfc_ext[:])
            nc.sync.dma_start(qs_t[:], qs_ext[:])
            nc.sync.dma_start(es_t[:], es_ext[:])

            ident = wpool.tile([128, 128], f32, tag="ident")
            ident8 = wpool.tile([128, 128], f8, tag="ident8")
            from concourse.masks import make_identity
            make_identity(nc, ident[:])
            nc.vector.tensor_copy(ident8[:], ident[:])
            eb_t = wpool.tile([128, 1], f32, tag="eb")
            nc.gpsimd.memset(eb_t[:], -3.0)

            # ---- persistent projected tensors ----
            qpT = ppool.tile([DKV, SQL], f16, tag="qpT")
            kpT = ppool.tile([DKV, S], f16, tag="kpT")
            vp_sb = ppool.tile([128, NKT, DV1], f16, tag="vp")
            eT = ppool.tile([128, NKT, SQL], f16, tag="eT")
            aoT = ppool.tile([DV1, SQL], f16, tag="aoT")
            rc_t = ppool.tile([128, SQL // 128], f32, tag="recip")

            with tc.tile_pool(name="pv_ps", bufs=1, space="PSUM") as pv_ps:
                pv_tiles = [pv_ps.tile([DV1, 512], f32, tag=f"pv{g}", name=f"pv{g}")
                            for g in range(2)]

                with tc.tile_pool(name="qk_in", bufs=2) as qk_in, \
                     tc.tile_pool(name="v_in", bufs=2) as v_in, \
                     tc.tile_pool(name="gm_in", bufs=3) as gm_in, \
                     tc.tile_pool(name="ps_proj", bufs=2, space="PSUM") as ps_proj, \
                     tc.tile_pool(name="ps_sc", bufs=2, space="PSUM") as ps_sc:

                    # Q projection: qpT[64, 1024] = sum_j Wq_j^T @ qT_j, scaled
                    q_sb = qk_in.tile([128, 4, SQL], f16, tag="qk")
                    nc.sync.dma_start(q_sb[:], qT_ext.rearrange("(j p) n -> p j n", p=128))
                    for c in range(SQL // 512):
                        pp = ps_proj.tile([128, 512], f32, tag="pp")
                        for j in range(4):
                            nc.tensor.matmul(pp[:DKV, :], wq_t[:, j, :DKV],
                                             q_sb[:, j, 512 * c:512 * (c + 1)],
                                             start=(j == 0), stop=(j == 3))
                        nc.vector.tensor_scalar(
                            out=qpT[:, 512 * c:512 * (c + 1)], in0=pp[:DKV, :],
                            scalar1=bq_t[:], scalar2=qs_t[:], op0=OP.add, op1=OP.mult)

                    # K projection: kpT[64, 2048], in 2 column chunks of 1024
                    for c2 in range(2):
                        k_sb = qk_in.tile([128, 4, SQL], f16, tag="qk")
                        nc.sync.dma_start(
                            k_sb[:], kT_ext[:, SQL * c2:SQL * (c2 + 1)]
                            .rearrange("(j p) n -> p j n", p=128))
                        for c in range(2):
                            pp = ps_proj.tile([128, 512], f32, tag="pp")
                            for j in range(4):
                                nc.tensor.matmul(pp[:DKV, :], wk_t[:, j, :DKV],
                                                 k_sb[:, j, 512 * c:512 * (c + 1)],
                                                 start=(j == 0), stop=(j == 3))
                            nc.vector.tensor_scalar(
                                out=kpT[:, 1024 * c2 + 512 * c:1024 * c2 + 512 * (c + 1)],
                                in0=pp[:DKV, :], scalar1=bk_t[:], scalar2=None, op0=OP.add)

                    # Phase B1 (+ V projection interleaved): per k-tile scores+exp
                    vt_sb = None
                    for kt in range(NKT):
                        if kt % 2 == 0:
                            gm_sb = gm_in.tile([128, 2, SQL], f8, tag="gm")
                            nc.gpsimd.dma_start(
                                gm_sb[:], gmT_ext[128 * kt:128 * (kt + 2), :]
                                .rearrange("(t p) q -> p t q", p=128))
                        if kt % 8 == 0:
                            vt_sb = v_in.tile([128, 4, SQL], f16, tag="vt")
                            nc.scalar.dma_start(
                                vt_sb[:], vT_ext[:, SQL * (kt // 8):SQL * (kt // 8 + 1)]
                                .rearrange("(j p) n -> p j n", p=128))

                        ps = ps_sc.tile([128, SQL], f32, tag="sc")
                        ksl = slice(128 * kt, 128 * (kt + 1))
                        for g in range(2):
                            qsl = slice(512 * g, 512 * (g + 1))
                            nc.tensor.matmul(ps[:, qsl], kpT[:, ksl], qpT[:, qsl],
                                             start=True, stop=False)
                            nc.tensor.matmul(ps[:, qsl], ident8[:],
                                             gm_sb[:, kt % 2, qsl],
                                             start=False, stop=True)
                        nc.scalar.activation(eT[:, kt, :], ps[:], AF.Exp,
                                             bias=eb_t[:], scale=es_t[:])

                        # V projection for this k-tile: vp[128, 65] (+ones col)
                        pv = ps_proj.tile([128, 512], f32, tag="pp")
                        vloc = kt % 8
                        for j in range(4):
                            nc.tensor.matmul(
                                pv[:, :DV1], vt_sb[:, j, 128 * vloc:128 * (vloc + 1)],
                                wv_t[:, j, :], start=(j == 0), stop=(j == 3))
                        nc.vector.tensor_tensor(out=vp_sb[:, kt, :], in0=pv[:, :DV1],
                                                in1=bv_t[:], op=OP.add)

                    # PV: oT_aug[65, 512] per q-group, accumulated over k-tiles
                    for g in range(2):
                        qsl = slice(512 * g, 512 * (g + 1))
                        for kt in range(NKT):
                            nc.tensor.matmul(pv_tiles[g][:], vp_sb[:, kt, :],
                                             eT[:, kt, qsl],
                                             start=(kt == 0), stop=(kt == NKT - 1))
                        nc.vector.tensor_copy(aoT[:, qsl], pv_tiles[g][:])

            # ---- FC + normalize + store ----
            with tc.tile_pool(name="out_sb", bufs=3) as out_pool, \
                 tc.tile_pool(name="ps_fc", bufs=2, space="PSUM") as ps_fc, \
                 tc.tile_pool(name="ps_rs", bufs=2, space="PSUM") as ps_rs:
                for qc in range(SQL // 128):
                    qsl = slice(128 * qc, 128 * (qc + 1))
                    fc = ps_fc.tile([128, D], f32, tag="fc")
                    rs = ps_rs.tile([128, 1], f32, tag="rs")
                    nc.tensor.matmul(fc[:], aoT[:, qsl], wfc_t[:],
                                     start=True, stop=True)
                    nc.tensor.matmul(rs[:], aoT[:, qsl], sel_t[:],
                                     start=True, stop=True)
                    nc.vector.reciprocal(rc_t[:, qc:qc + 1], rs[:])
                    o_sb = out_pool.tile([128, D], f32, tag="osb")
                    nc.vector.scalar_tensor_tensor(
                        out=o_sb[:], in0=fc[:], scalar=rc_t[:, qc:qc + 1],
                        in1=bfc_t[:], op0=OP.mult, op1=OP.add)
                    nc.sync.dma_start(out_ext[qsl, :], o_sb[:])

    nc.finalize()
    return nc


_cache = {}


def kernel(**inputs):
    from concourse.bass_utils import run_bass_kernel_spmd

    q = np.asarray(inputs["q"], np.float32)
    k = np.asarray(inputs["k"], np.float32)
    v = np.asarray(inputs["v"], np.float32)
    gb = np.asarray(inputs["g_bias"], np.float32)
    mask = np.asarray(inputs["mask"]).astype(bool)
    tau = float(np.asarray(inputs["tau"]))

    if "nc" not in _cache:
        _cache["nc"] = _build()
    nc = _cache["nc"]

    in_maps = build_in_maps(inputs, q, k, v, gb, mask, tau)
    res = run_bass_kernel_spmd(nc, in_maps, list(range(N_CORES)))
    out = np.empty((B, S, D), np.float32)
    for c in range(N_CORES):
        b, h = divmod(c, 2)
        out[b, h * SQL:(h + 1) * SQL] = res.results[c]["out"]
    return out


def build_in_maps(inputs, q, k, v, gb, mask, tau):
    f16 = np.float16
    f8 = ml_dtypes.float8_e5m2
    wv_aug = np.zeros((D, DV1), np.float32)
    wv_aug[:, :DKV] = np.asarray(inputs["Wv"], np.float32)
    wfc_aug = np.zeros((DV1, D), np.float32)
    wfc_aug[:DKV] = np.asarray(inputs["Wfc"], np.float32)
    sel = np.zeros((DV1, 1), np.float32)
    sel[DKV, 0] = 1.0
    bv_aug = np.zeros((128, DV1), np.float32)
    bv_aug[:, :DKV] = np.asarray(inputs["bv"], np.float32)
    bv_aug[:, DKV] = 1.0
    shared = {
        "Wq": np.asarray(inputs["Wq"], f16),
        "Wk": np.asarray(inputs["Wk"], f16),
        "Wva": wv_aug.astype(f16),
        "Wfca": wfc_aug.astype(f16),
        "sel": sel.astype(f16),
        "bq": np.asarray(inputs["bq"], np.float32).reshape(DKV, 1).copy(),
        "bk": np.asarray(inputs["bk"], np.float32).reshape(DKV, 1).copy(),
        "bvb": bv_aug,
        "bfcb": np.broadcast_to(np.asarray(inputs["bfc"], np.float32), (128, D)).copy(),
        "qscale": np.full((DKV, 1), (2.0 * tau * tau) / 8.0, np.float32),
        "escale": np.full((128, 1), 1.0 / (2.0 * tau * tau), np.float32),
    }
    mask = np.asarray(mask).astype(bool)
    in_maps = []
    for c in range(N_CORES):
        b, h = divmod(c, 2)
        sl = slice(h * SQL, (h + 1) * SQL)
        gmT = np.where(mask[b, sl].T, np.float32(MASK_VAL), gb[b, sl].T)
        in_maps.append({
            "qT": np.ascontiguousarray(q[b, sl].T.astype(f16)),
            "kT": np.ascontiguousarray(k[b].T.astype(f16)),
            "vT": np.ascontiguousarray(v[b].T.astype(f16)),
            "gmT": np.ascontiguousarray(gmT.astype(f8)),
            **shared,
        })
    return in_maps
